# revision 1
# baseline (speedup 1.0000x reference)
"""Bass/Trainium2 kernel for nn_CustomLoss_43834436223359 (retrieval_knn).

Strategy:
  - The only heavy part is the brute-force KNN scan: d2 = ||Tq - X||^2 over
    [B=256, N=200000] and top-50 per row.  X is sharded row-wise across 8
    NeuronCores (25000 rows of X per core, padded to 17*1536).
  - On device, each core computes scores = sum_{d<127} Tq_d * X_d - 0.5*||x||^2
    via PE matmuls (the per-column bias -0.5*||x||^2 rides in contraction row
    127 with a matching 1.0 in the query operand; dropping data dim 127 only
    perturbs the ranking, never the final values).  Ranking by that score
    descending == ranking by d2 ascending up to a small, coverage-covered
    perturbation.
  - Selection is DVE-bound (max/max_index run at 1 elem/cycle), so scores are
    first reduced 8:1 with three strided pairwise-max passes
    (1536->768->384->192); max8/max_index then pick the top-8 group maxima
    per 1536-wide segment.  Each winner covers a contiguous 8-column group.
  - Host expands the 8*17*8 winner groups, re-scores candidates exactly in
    f64, picks the true top-50, then computes the tiny MMD / union-KL / reg /
    anchor terms in numpy.  Device precision only affects candidate
    *coverage*, which has enormous margin.
"""

import numpy as np
import ml_dtypes

BF16 = ml_dtypes.bfloat16

B, D, N, NQ, K = 256, 128, 200000, 10000, 50
NCORES = 8
SHARD = 25000
SEG1 = 1536                   # PSUM chunk: 3 banks
NSEG = 17
PADDED = NSEG * SEG1          # 26112
NCAND = NSEG * 8              # 136 winners per core per row
GRP = 16                      # columns covered by one winner
RED = SEG1 // GRP             # 96: width of the final max8 scan (= col stride)
TAU = 0.1
EPS = 1e-8
ALPHA, BETA, LAMB, GAMMA = 1.0, 1.0, 1e-4, 1.0

_cache = {}
last_results = None


def _patch_tail_drain():
    """Split the TileContext tail drain into one drain per pending proc:
    the stock implementation attaches a wait for EVERY proc in the global
    clock to a single Drain, overflowing the ISA's sync-wait slots."""
    import concourse.tile as tile
    from concourse.vector_clock import ScopedClock, VectorClock

    if getattr(tile.TileContext, "_ant_split_drain", False):
        return

    def _drain_and_barrier(self, tick_clock, wait_clock):
        vc = tick_clock.global_clock
        for proc in range(len(vc)):
            t = vc[proc]
            if t > 0:
                drain_inst = self.nc.sync.drain()
                sub = [0] * len(vc)
                sub[proc] = t
                wait_clock.add_sem_waits(
                    drain_inst.ins, ScopedClock({None: VectorClock(sub)})
                )
        self.nc.all_engine_barrier()
        assert self.sems is not None
        popped = self.nc._tile_sem_poison_stack.pop()
        assert popped is self._sem_poison
        self.nc.clear_and_free_semaphores(list(self.sems.allocated().values()))
        self.nc.all_engine_barrier()

    tile.TileContext._drain_and_barrier = _drain_and_barrier
    tile.TileContext._ant_split_drain = True


def _split_multi_waits(nc, max_waits=1):
    """Walrus legality pass: TRN2 instruction structs carry very few sync-wait
    slots (1 for Matmult/DMA/Activation/TensorTensor).  Hoist excess waits
    onto same-engine NoOps inserted right before the instruction — the engine
    queue stalls on the NoOp first, preserving semantics exactly."""
    import concourse.mybir as mybir
    f = nc.m.functions[0]
    for blk in f.blocks:
        insts = blk.instructions
        out = []
        changed = False
        for inst in insts:
            si = getattr(inst, "sync_info", None)
            if si is not None and len(si.on_wait) > max_waits:
                waits = list(si.on_wait)
                for w in waits[:-max_waits]:
                    nop = mybir.InstNoOp(name=f"I-wsplit-{nc.next_id()}")
                    nop.engine = inst.engine
                    nop.sync_info = mybir.SyncInfo(on_wait=[w], on_update=[])
                    out.append(nop)
                inst.sync_info = mybir.SyncInfo(
                    on_wait=waits[-max_waits:], on_update=list(si.on_update))
                changed = True
            out.append(inst)
        if changed:
            blk.instructions = out


def _build_bass(trace_sim=False):
    import concourse.bass as bass
    import concourse.mybir as mybir
    from concourse.tile import TileContext

    _patch_tail_drain()

    nc = bass.Bass()
    lhs_d = nc.dram_tensor("lhs", [128, 256], mybir.dt.bfloat16, kind="ExternalInput")
    xt_d = nc.dram_tensor("xt", [128, PADDED], mybir.dt.bfloat16, kind="ExternalInput")
    cval_d = nc.dram_tensor("cval", [256, NCAND], mybir.dt.float32, kind="ExternalOutput")
    cidx_d = nc.dram_tensor("cidx", [256, NCAND], mybir.dt.uint16, kind="ExternalOutput")

    # Only 8 HW DGE queues exist and queue assignment is a global round-robin;
    # a DMA landing on a queue that already carried one picks up a ring-order
    # wait, and DMA instructions fit only ONE sync wait.  So: at most 8 DMAs.
    DMA_WIDTHS = [1536, 1536] + [3072] * 6 + [4608]

    with TileContext(nc, trace_sim=trace_sim) as tc:
        with (
            tc.tile_pool(name="xin", bufs=1) as xin_pool,
            tc.tile_pool(name="ps", bufs=1, space="PSUM") as psum_pool,
            tc.tile_pool(name="misc", bufs=1) as misc_pool,
        ):
            # All tiles are allocated ONCE and reused by reference: pool-slot
            # recycling would create fresh tile objects whose WAW deps get
            # semaphores even on the same engine, and TRN2's Matmult ISA
            # struct carries at most ONE sync wait.  With fixed tiles, PE's
            # same-tile WAW rides program order; each real matmul then waits
            # only on the DVE reader of its PSUM tile.  A tiny PE "absorber"
            # matmul touches each freshly-DMA'd xt tile first, taking the DMA
            # wait so real matmuls never see it.
            ones_sb = misc_pool.tile([128, 8], mybir.dt.bfloat16, tag="ones")
            nc.vector.memset(ones_sb[:], 1.0)
            ones_f32 = misc_pool.tile([128, 8], mybir.dt.float32, tag="onesf")
            nc.vector.memset(ones_f32[:], 1.0)
            nop_ps = psum_pool.tile([1, 8], mybir.dt.float32, tag="nop")
            nc.tensor.matmul(nop_ps[:], ones_sb[:, :1], ones_sb[:, :8],
                             start=True, stop=True)
            nc.tensor.matmul(nop_ps[:], ones_f32[:, :1], ones_f32[:, :8],
                             start=True, stop=True)

            lhs_sb = misc_pool.tile([128, 256], mybir.dt.bfloat16, tag="lhs")
            nc.sync.dma_start(out=lhs_sb[:], in_=lhs_d[:])
            cval_sb = misc_pool.tile([128, 2 * NCAND], mybir.dt.float32, tag="cv")
            cidx_sb = misc_pool.tile([128, 2 * NCAND], mybir.dt.uint16, tag="ci")
            xt_tiles = [xin_pool.tile([128, DMA_WIDTHS[i]], mybir.dt.bfloat16,
                                      name=f"xt{i}", tag=f"xt{i}")
                        for i in range(len(DMA_WIDTHS))]
            ps_tiles = [psum_pool.tile([128, SEG1], mybir.dt.float32,
                                       name=f"ps{i}", tag=f"ps{i}") for i in range(2)]
            r1s = [misc_pool.tile([128, SEG1 // 2], mybir.dt.float32,
                                  name=f"r1{i}", tag=f"r1{i}") for i in range(2)]
            r2s = [misc_pool.tile([128, SEG1 // 4], mybir.dt.float32,
                                  name=f"r2{i}", tag=f"r2{i}") for i in range(2)]
            r3s = [misc_pool.tile([128, SEG1 // 8], mybir.dt.float32,
                                  name=f"r3{i}", tag=f"r3{i}") for i in range(2)]
            r4s = [misc_pool.tile([128, RED], mybir.dt.float32,
                                  name=f"r4{i}", tag=f"r4{i}") for i in range(2)]
            odds = [misc_pool.tile([128, SEG1 // 2], mybir.dt.float32,
                                   name=f"odd{i}", tag=f"odd{i}") for i in range(2)]
            scr = misc_pool.tile([1, 8], mybir.dt.float32, tag="scr")

            seg = 0
            cg = 0
            for t, w in enumerate(DMA_WIDTHS):
                xt_sb = xt_tiles[t]
                nc.sync.dma_start(out=xt_sb[:],
                                  in_=xt_d[:, sum(DMA_WIDTHS[:t]):sum(DMA_WIDTHS[:t]) + w])
                nc.tensor.matmul(nop_ps[:], ones_sb[:, :1], xt_sb[:, :8],
                                 start=True, stop=True)
                for j in range(w // SEG1):
                    for g in range(2):
                        par = cg % 2
                        ps = ps_tiles[par]
                        r1, r2, r3, r4 = r1s[par], r2s[par], r3s[par], r4s[par]
                        odd_sb = odds[par]
                        # PE absorber: observing r1's last writer (DVE) covers
                        # this PSUM tile's pending DVE reader, so the real
                        # matmuls below carry only the ACT-copy WAR wait.
                        if cg >= 2:
                            nc.tensor.matmul(nop_ps[:], ones_f32[:, :1],
                                             r1[:, :8], start=True, stop=True)
                        cg += 1
                        for m in range(SEG1 // 512):
                            nc.tensor.matmul(
                                ps[:, m * 512:(m + 1) * 512],
                                lhs_sb[:, g * 128:(g + 1) * 128],
                                xt_sb[:, j * SEG1 + m * 512: j * SEG1 + (m + 1) * 512],
                                start=True, stop=True)
                        # ACT stages the second half to SBUF (one PSUM read
                        # port on DVE); DVE mini-copy absorbs the ACT dep so
                        # tensor_max keeps a single wait.
                        half = SEG1 // 2
                        nc.scalar.copy(odd_sb[:], ps[:, half:])
                        nc.vector.tensor_max(r1[:], ps[:, :half], odd_sb[:])
                        q = SEG1 // 4
                        nc.vector.tensor_max(r2[:], r1[:, :q], r1[:, q:])
                        nc.vector.tensor_max(r3[:], r2[:, :q // 2], r2[:, q // 2:])
                        nc.vector.tensor_max(r4[:], r3[:, :RED], r3[:, RED:])
                        o = g * NCAND + seg * 8
                        nc.vector.max(out=cval_sb[:, o:o + 8], in_=r4[:])
                        nc.vector.max_index(cidx_sb[:, o:o + 8],
                                            cval_sb[:, o:o + 8], r4[:])
                    seg += 1
            nc.sync.dma_start(
                out=cval_d[:, :].rearrange("(g p) n -> p g n", g=2),
                in_=cval_sb[:, :].rearrange("p (g n) -> p g n", g=2))
            nc.sync.dma_start(
                out=cidx_d[:, :].rearrange("(g p) n -> p g n", g=2),
                in_=cidx_sb[:, :].rearrange("p (g n) -> p g n", g=2))
    _split_multi_waits(nc)
    return nc


def _device_candidates(Tq32, X32, xsq64):
    """Run the 8-core SPMD kernel; return per-row winner groups.

    Returns (gstart, istart, vals): global start column, in-core start column
    and value for each of the 8*NCAND winners per row; each winner covers
    columns [gstart, gstart+GRP).
    """
    global last_results
    from concourse.bass_utils import run_bass_kernel_spmd

    if "nc" not in _cache:
        _cache["nc"] = _build_bass()
    nc = _cache["nc"]

    lhs = np.zeros([128, 256], np.float32)
    lhs[:127, :] = Tq32.T[:127, :]
    lhs[127, :] = 1.0
    lhs = lhs.astype(BF16)

    in_maps = []
    for c in range(NCORES):
        xt = np.zeros([128, PADDED], np.float32)
        sl = X32[c * SHARD:(c + 1) * SHARD]
        xt[:127, :SHARD] = sl.T[:127, :]
        xt[127, :SHARD] = (-0.5 * xsq64[c * SHARD:(c + 1) * SHARD]).astype(np.float32)
        xt[127, SHARD:] = -1e30
        in_maps.append({"lhs": lhs, "xt": xt.astype(BF16)})

    import time
    t0 = time.perf_counter()
    last_results = run_bass_kernel_spmd(nc, in_maps, core_ids=list(range(NCORES)))
    _cache["spmd_wall_s"] = time.perf_counter() - t0
    results = last_results.results

    vals = np.concatenate([np.asarray(r["cval"], np.float32) for r in results], axis=1)
    locs = np.concatenate([np.asarray(r["cidx"], np.int64) for r in results], axis=1)
    seg_of = np.tile(np.arange(NCAND) // 8 * SEG1, NCORES)      # [8*NCAND]
    core_of = np.repeat(np.arange(NCORES) * SHARD, NCAND)
    # winner at position p covers in-chunk columns {p + RED*k, k<GRP}
    istart = seg_of[None, :] + locs
    gstart = core_of[None, :] + istart
    return gstart, istart, vals


def _topk_exact(Tq64, X64, gstart, istart, vals, k=K, prefilter=150):
    """Exact top-k per row: expand winner groups, re-score in f64."""
    Bn = Tq64.shape[0]
    out = np.empty((Bn, k), np.int64)
    neg = np.where(vals > -1e29, vals, -np.inf)
    d = RED * np.arange(GRP)
    for i in range(Bn):
        w = np.argpartition(-neg[i], prefilter)[:prefilter]
        cc = (gstart[i, w][:, None] + d[None, :]).ravel()
        ok = ((istart[i, w][:, None] + d[None, :]).ravel() < SHARD)
        cc = np.unique(cc[ok])
        diff = X64[cc] - Tq64[i]
        d2 = np.einsum("ij,ij->i", diff, diff)
        order = np.lexsort((cc, d2))
        out[i] = cc[order[:k]]
    return out


def _sqdist(A, Bm):
    d2 = (A * A).sum(1)[:, None] + (Bm * Bm).sum(1)[None, :] - 2.0 * (A @ Bm.T)
    return np.maximum(d2, 0.0)


def _host_loss(q_batch, X, W, b, pre_weights, pre_indices, q_indices, idx, post_idx):
    """Mirror of reference() in numpy f64, given the KNN indices."""
    Tq = q_batch @ W.T + b
    # ---- MMD ----
    s, t = Tq, X[idx]
    comb = np.concatenate([s, t], 0)
    sigma_sq = np.median(_sqdist(comb, comb)) / 2.0
    if sigma_sq < 1e-6:
        sigma_sq = 1.0
    g = 1.0 / (sigma_sq + EPS)
    kxx = np.exp(-g * _sqdist(s, s)).mean()
    kyy = np.exp(-g * _sqdist(t, t)).mean()
    kxy = np.exp(-g * _sqdist(s, t)).mean()
    loss_dist = max(kxx + kyy - 2.0 * kxy, 0.0)
    # ---- KNN softmax over exact l2 of selected neighbors ----
    Xn = X[post_idx]                                   # [B, K, d]
    l2 = ((Tq[:, None, :] - Xn) ** 2).sum(-1)          # [B, K]
    z = -l2 / TAU
    z = z - z.max(1, keepdims=True)
    ez = np.exp(z)
    post_w = ez / ez.sum(1, keepdims=True)
    # ---- union-KL ----
    pre_i = pre_indices[q_indices]                     # [B, K]
    pre_w = pre_weights[q_indices]                     # [B, K]
    cat = np.concatenate([pre_i, post_idx], axis=1)    # [B, 2K]
    mult = (cat[:, :, None] == cat[:, None, :]).sum(-1).astype(np.float64)
    p_raw = np.einsum("bmk,bk->bm",
                      (cat[:, :, None] == pre_i[:, None, :]).astype(np.float64), pre_w)
    q_raw = np.einsum("bmk,bk->bm",
                      (cat[:, :, None] == post_idx[:, None, :]).astype(np.float64), post_w)
    p_c = np.maximum(p_raw, EPS)
    q_c = np.maximum(q_raw, EPS)
    p = p_c / (p_c / mult).sum(1, keepdims=True)
    q = q_c / (q_c / mult).sum(1, keepdims=True)
    kl = ((p * (np.log(p) - np.log(q))) / mult).sum(1)
    loss_knn = kl.mean()
    # ---- reg & anchor ----
    loss_reg = 0.5 * ((W ** 2).sum() + (b ** 2).sum())
    loss_anchor = ((Tq - q_batch) ** 2).sum(1).mean()
    total = ALPHA * loss_dist + BETA * loss_knn + LAMB * loss_reg + GAMMA * loss_anchor
    return np.stack([total, loss_dist, loss_knn, loss_anchor]).astype(np.float32)


def kernel(q_batch, X, W, b, pre_weights, pre_indices, q_indices, idx):
    q_batch = np.asarray(q_batch, np.float32)
    X32 = np.asarray(X, np.float32)
    W32 = np.asarray(W, np.float32)
    b32 = np.asarray(b, np.float32)
    pre_weights = np.asarray(pre_weights, np.float64)
    pre_indices = np.asarray(pre_indices, np.int64)
    q_indices = np.asarray(q_indices, np.int64)
    idx = np.asarray(idx, np.int64)

    Tq32 = q_batch @ W32.T + b32
    X64 = X32.astype(np.float64)
    Tq64 = Tq32.astype(np.float64)
    xsq64 = (X64 * X64).sum(1)

    gstart, istart, vals = _device_candidates(Tq32, X32, xsq64)
    post_idx = _topk_exact(Tq64, X64, gstart, istart, vals)

    return _host_loss(q_batch.astype(np.float64), X64, W32.astype(np.float64),
                      b32.astype(np.float64), pre_weights, pre_indices,
                      q_indices, idx, post_idx)



# revision 3
# speedup vs baseline: 1.4889x; 1.4889x over previous
"""Bass/Trainium2 kernel for nn_CustomLoss_43834436223359 (retrieval_knn).

Strategy (v2):
  - Heavy part: brute-force KNN scan d2 = ||Tq - X||^2 over [B=256, N=200000]
    with top-50 per row.  X sharded row-wise across 8 cores (25000 cols each,
    padded to 13*2048 = 26624).
  - Per core, scores = sum_{d<127} Tq_d * X_d - 0.5*||x||^2 via PE matmuls
    (bias rides contraction row 127; ranking by score desc == d2 asc).
  - Selection is engine-balanced instead of a deep DVE tree:
      per 2048-col PSUM instance (4 matmuls, f32):
        ACT: one copy ps[:, 928:2048] -> bf16 staged tile     (~1.15us)
        DVE: one tensor_max(ps[:, 0:928], staged[0:928])      (~1.14us)
             -> bf16 "entries" (each covers 2 columns)
      The staged tail [928:1120) goes to HBM raw (1 column per value).
    No on-device top-k at all: entries + raw scores are DMA'd out and the
    host does argpartition + exact f64 rescoring (device values only gate
    candidate *coverage*, which has large margin).
  - DMA engine parallelism: X loads are issued from SP, entry/raw exports
    from the (otherwise idle) GPSIMD engine; transfers occupy the issuing
    engine's timeline, so they overlap compute on ACT/DVE/PE.
"""

import numpy as np
import ml_dtypes

BF16 = ml_dtypes.bfloat16

B, D, N, NQ, K = 256, 128, 200000, 10000, 50
NCORES = 8
SHARD = 25000
SEG = 2048                    # PSUM instance: 4 banks f32
NSEG = 13
PADDED = NSEG * SEG           # 26624
W = 928                       # columns pair-merged by DVE per instance
RAW = SEG - 2 * W             # 192 raw (single-column) scores per instance
STG = SEG - W                 # 1120 staged bf16 columns per instance
NINST = 2 * NSEG              # 26 instances (2 query groups x 13 segments)
TAU = 0.1
EPS = 1e-8
ALPHA, BETA, LAMB, GAMMA = 1.0, 1.0, 1e-4, 1.0

# host-side selection breadth
T_ENT = 2600                  # top entries taken per row (each -> 2 cols)
T_RAW = 500                   # top raw scores taken per row (1 col each)

_cache = {}
last_results = None


def _patch_tail_drain():
    """Split the TileContext tail drain into one drain per pending proc:
    the stock implementation attaches a wait for EVERY proc in the global
    clock to a single Drain, overflowing the ISA's sync-wait slots."""
    import concourse.tile as tile
    from concourse.vector_clock import ScopedClock, VectorClock

    if getattr(tile.TileContext, "_ant_split_drain", False):
        return

    def _drain_and_barrier(self, tick_clock, wait_clock):
        vc = tick_clock.global_clock
        for proc in range(len(vc)):
            t = vc[proc]
            if t > 0:
                drain_inst = self.nc.sync.drain()
                sub = [0] * len(vc)
                sub[proc] = t
                wait_clock.add_sem_waits(
                    drain_inst.ins, ScopedClock({None: VectorClock(sub)})
                )
        self.nc.all_engine_barrier()
        assert self.sems is not None
        popped = self.nc._tile_sem_poison_stack.pop()
        assert popped is self._sem_poison
        self.nc.clear_and_free_semaphores(list(self.sems.allocated().values()))
        self.nc.all_engine_barrier()

    tile.TileContext._drain_and_barrier = _drain_and_barrier
    tile.TileContext._ant_split_drain = True


def _split_multi_waits(nc, max_waits=1):
    """Walrus legality pass: TRN2 instruction structs carry very few sync-wait
    slots (1 for Matmult/DMA/Activation/TensorTensor).  Hoist excess waits
    onto same-engine NoOps inserted right before the instruction — the engine
    queue stalls on the NoOp first, preserving semantics exactly."""
    import concourse.mybir as mybir
    f = nc.m.functions[0]
    for blk in f.blocks:
        insts = blk.instructions
        out = []
        changed = False
        for inst in insts:
            si = getattr(inst, "sync_info", None)
            if si is not None and len(si.on_wait) > max_waits:
                waits = list(si.on_wait)
                for w in waits[:-max_waits]:
                    nop = mybir.InstNoOp(name=f"I-wsplit-{nc.next_id()}")
                    nop.engine = inst.engine
                    nop.sync_info = mybir.SyncInfo(on_wait=[w], on_update=[])
                    out.append(nop)
                inst.sync_info = mybir.SyncInfo(
                    on_wait=waits[-max_waits:], on_update=list(si.on_update))
                changed = True
            out.append(inst)
        if changed:
            blk.instructions = out


def _build_bass(trace_sim=False):
    import concourse.bass as bass
    import concourse.mybir as mybir
    from concourse.tile import TileContext

    _patch_tail_drain()

    nc = bass.Bass()
    lhs_d = nc.dram_tensor("lhs", [128, 256], mybir.dt.bfloat16, kind="ExternalInput")
    xt_d = nc.dram_tensor("xt", [128, PADDED], mybir.dt.bfloat16, kind="ExternalInput")
    ent_d = nc.dram_tensor("ent", [128, NINST * W], mybir.dt.bfloat16,
                           kind="ExternalOutput")
    raw_d = nc.dram_tensor("raw", [128, NINST * RAW], mybir.dt.bfloat16,
                           kind="ExternalOutput")

    with TileContext(nc, trace_sim=trace_sim) as tc:
        with (
            tc.tile_pool(name="xin", bufs=1) as xin_pool,
            tc.tile_pool(name="ps", bufs=1, space="PSUM") as psum_pool,
            tc.tile_pool(name="misc", bufs=1) as misc_pool,
        ):
            lhs_sb = misc_pool.tile([128, 256], mybir.dt.bfloat16, tag="lhs")
            nc.sync.dma_start(out=lhs_sb[:], in_=lhs_d[:])
            xt_tiles = [xin_pool.tile([128, SEG], mybir.dt.bfloat16,
                                      name=f"xt{s}", tag=f"xt{s}")
                        for s in range(NSEG)]
            for s in range(NSEG):
                nc.sync.dma_start(out=xt_tiles[s][:],
                                  in_=xt_d[:, s * SEG:(s + 1) * SEG])
            ps_tiles = [psum_pool.tile([128, SEG], mybir.dt.float32,
                                       name=f"ps{i}", tag=f"ps{i}") for i in range(2)]
            stg_all = misc_pool.tile([128, NINST * STG], mybir.dt.bfloat16, tag="stg")
            ent_all = misc_pool.tile([128, NINST * W], mybir.dt.bfloat16, tag="ent")

            for i in range(NINST):
                s, g = i // 2, i % 2
                ps = ps_tiles[g]
                xt_sb = xt_tiles[s]
                for m in range(SEG // 512):
                    nc.tensor.matmul(
                        ps[:, m * 512:(m + 1) * 512],
                        lhs_sb[:, g * 128:(g + 1) * 128],
                        xt_sb[:, m * 512:(m + 1) * 512],
                        start=True, stop=True)
                o = i * STG
                nc.scalar.copy(stg_all[:, o:o + STG], ps[:, W:SEG])
                nc.vector.tensor_max(ent_all[:, i * W:(i + 1) * W],
                                     ps[:, 0:W], stg_all[:, o:o + W])
                if g == 1:
                    nc.gpsimd.dma_start(
                        out=ent_d[:, (i - 1) * W:(i + 1) * W],
                        in_=ent_all[:, (i - 1) * W:(i + 1) * W])
                if i % 4 == 3 or i == NINST - 1:
                    i0 = (i // 4) * 4
                    k = i - i0 + 1
                    nc.gpsimd.dma_start(
                        out=raw_d[:, i0 * RAW:(i + 1) * RAW].rearrange(
                            "p (k r) -> p k r", k=k),
                        in_=stg_all[:, i0 * STG:(i + 1) * STG].rearrange(
                            "p (k t) -> p k t", k=k)[:, :, W:STG])
    _split_multi_waits(nc)
    return nc


def _device_scores(Tq32, X32, xsq64):
    """Run the 8-core SPMD kernel; return per-core (ent, raw) bf16 arrays."""
    global last_results
    from concourse.bass_utils import run_bass_kernel_spmd

    if "nc" not in _cache:
        _cache["nc"] = _build_bass()
    nc = _cache["nc"]

    lhs = np.zeros([128, 256], np.float32)
    lhs[:127, :] = Tq32.T[:127, :]
    lhs[127, :] = 1.0
    lhs = lhs.astype(BF16)

    in_maps = []
    for c in range(NCORES):
        xt = np.zeros([128, PADDED], np.float32)
        sl = X32[c * SHARD:(c + 1) * SHARD]
        xt[:127, :SHARD] = sl.T[:127, :]
        xt[127, :SHARD] = (-0.5 * xsq64[c * SHARD:(c + 1) * SHARD]).astype(np.float32)
        xt[127, SHARD:] = -1e30
        in_maps.append({"lhs": lhs, "xt": xt.astype(BF16)})

    import time
    t0 = time.perf_counter()
    last_results = run_bass_kernel_spmd(nc, in_maps, core_ids=list(range(NCORES)))
    _cache["spmd_wall_s"] = time.perf_counter() - t0
    results = last_results.results

    ents = [np.asarray(r["ent"]) for r in results]   # [128, NINST*W] bf16
    raws = [np.asarray(r["raw"]) for r in results]   # [128, NINST*RAW] bf16
    return ents, raws


def _entry_colmaps():
    """Column maps, core-local.  ent entry (s, j) covers cols
    {s*SEG + j, s*SEG + W + j}; raw (s, k) covers col s*SEG + 2W + k."""
    s = np.arange(NSEG)
    j = np.arange(W)
    e0 = (s[:, None] * SEG + j[None, :]).ravel()          # [NSEG*W]
    e1 = e0 + W
    k = np.arange(RAW)
    r0 = (s[:, None] * SEG + 2 * W + k[None, :]).ravel()  # [NSEG*RAW]
    return e0, e1, r0


def _topk_exact(Tq64, X64, ents, raws, k=K):
    """Exact top-k per row from device candidate scores."""
    e0, e1, r0 = _entry_colmaps()
    # per-core global column offsets; pad locals (>= SHARD) masked per map
    ecols0 = np.concatenate([c * SHARD + e0 for c in range(NCORES)])
    ecols1 = np.concatenate([c * SHARD + e1 for c in range(NCORES)])
    rcols = np.concatenate([c * SHARD + r0 for c in range(NCORES)])
    e0valid = np.tile(e0 < SHARD, NCORES)
    e1valid = np.tile(e1 < SHARD, NCORES)
    rvalid = np.tile(r0 < SHARD, NCORES)
    evalid = e0valid | e1valid

    out = np.empty((B, k), np.int64)
    for q in range(B):
        g, p = divmod(q, 128)
        ev = np.concatenate(
            [e[p].reshape(NINST, W)[g::2].ravel() for e in ents]).astype(np.float32)
        rv = np.concatenate(
            [r[p].reshape(NINST, RAW)[g::2].ravel() for r in raws]).astype(np.float32)
        ev[~evalid] = -np.inf
        rv[~rvalid] = -np.inf
        te = np.argpartition(-ev, T_ENT)[:T_ENT]
        tr = np.argpartition(-rv, T_RAW)[:T_RAW]
        cc = np.concatenate([ecols0[te][e0valid[te]],
                             ecols1[te][e1valid[te]],
                             rcols[tr][rvalid[tr]]])
        cc = np.unique(cc)
        diff = X64[cc] - Tq64[q]
        d2 = np.einsum("ij,ij->i", diff, diff)
        order = np.lexsort((cc, d2))
        out[q] = cc[order[:k]]
    return out


def _sqdist(A, Bm):
    d2 = (A * A).sum(1)[:, None] + (Bm * Bm).sum(1)[None, :] - 2.0 * (A @ Bm.T)
    return np.maximum(d2, 0.0)


def _host_loss(q_batch, X, Wm, b, pre_weights, pre_indices, q_indices, idx, post_idx):
    """Mirror of reference() in numpy f64, given the KNN indices."""
    Tq = q_batch @ Wm.T + b
    s, t = Tq, X[idx]
    comb = np.concatenate([s, t], 0)
    sigma_sq = np.median(_sqdist(comb, comb)) / 2.0
    if sigma_sq < 1e-6:
        sigma_sq = 1.0
    g = 1.0 / (sigma_sq + EPS)
    kxx = np.exp(-g * _sqdist(s, s)).mean()
    kyy = np.exp(-g * _sqdist(t, t)).mean()
    kxy = np.exp(-g * _sqdist(s, t)).mean()
    loss_dist = max(kxx + kyy - 2.0 * kxy, 0.0)
    Xn = X[post_idx]                                   # [B, K, d]
    l2 = ((Tq[:, None, :] - Xn) ** 2).sum(-1)          # [B, K]
    z = -l2 / TAU
    z = z - z.max(1, keepdims=True)
    ez = np.exp(z)
    post_w = ez / ez.sum(1, keepdims=True)
    pre_i = pre_indices[q_indices]                     # [B, K]
    pre_w = pre_weights[q_indices]                     # [B, K]
    cat = np.concatenate([pre_i, post_idx], axis=1)    # [B, 2K]
    mult = (cat[:, :, None] == cat[:, None, :]).sum(-1).astype(np.float64)
    p_raw = np.einsum("bmk,bk->bm",
                      (cat[:, :, None] == pre_i[:, None, :]).astype(np.float64), pre_w)
    q_raw = np.einsum("bmk,bk->bm",
                      (cat[:, :, None] == post_idx[:, None, :]).astype(np.float64), post_w)
    p_c = np.maximum(p_raw, EPS)
    q_c = np.maximum(q_raw, EPS)
    p = p_c / (p_c / mult).sum(1, keepdims=True)
    q = q_c / (q_c / mult).sum(1, keepdims=True)
    kl = ((p * (np.log(p) - np.log(q))) / mult).sum(1)
    loss_knn = kl.mean()
    loss_reg = 0.5 * ((Wm ** 2).sum() + (b ** 2).sum())
    loss_anchor = ((Tq - q_batch) ** 2).sum(1).mean()
    total = ALPHA * loss_dist + BETA * loss_knn + LAMB * loss_reg + GAMMA * loss_anchor
    return np.stack([total, loss_dist, loss_knn, loss_anchor]).astype(np.float32)


def kernel(q_batch, X, W, b, pre_weights, pre_indices, q_indices, idx):
    q_batch = np.asarray(q_batch, np.float32)
    X32 = np.asarray(X, np.float32)
    W32 = np.asarray(W, np.float32)
    b32 = np.asarray(b, np.float32)
    pre_weights = np.asarray(pre_weights, np.float64)
    pre_indices = np.asarray(pre_indices, np.int64)
    q_indices = np.asarray(q_indices, np.int64)
    idx = np.asarray(idx, np.int64)

    Tq32 = q_batch @ W32.T + b32
    X64 = X32.astype(np.float64)
    Tq64 = Tq32.astype(np.float64)
    xsq64 = (X64 * X64).sum(1)

    ents, raws = _device_scores(Tq32, X32, xsq64)
    post_idx = _topk_exact(Tq64, X64, ents, raws)

    return _host_loss(q_batch.astype(np.float64), X64, W32.astype(np.float64),
                      b32.astype(np.float64), pre_weights, pre_indices,
                      q_indices, idx, post_idx)


# revision 7
# speedup vs baseline: 1.9248x; 1.2927x over previous
"""Bass/Trainium2 kernel for nn_CustomLoss_43834436223359 (retrieval_knn).

Strategy (v2):
  - Heavy part: brute-force KNN scan d2 = ||Tq - X||^2 over [B=256, N=200000]
    with top-50 per row.  X sharded row-wise across 8 cores (25000 cols each,
    padded to 13*2048 = 26624).
  - Per core, scores = sum_{d<127} Tq_d * X_d - 0.5*||x||^2 via PE matmuls
    (bias rides contraction row 127; ranking by score desc == d2 asc).
  - Selection is engine-balanced instead of a deep DVE tree:
      per 2048-col PSUM instance (4 matmuls, f32):
        ACT: one copy ps[:, 928:2048] -> bf16 staged tile     (~1.15us)
        DVE: one tensor_max(ps[:, 0:928], staged[0:928])      (~1.14us)
             -> bf16 "entries" (each covers 2 columns)
      The staged tail [928:1120) goes to HBM raw (1 column per value).
    No on-device top-k at all: entries + raw scores are DMA'd out and the
    host does argpartition + exact f64 rescoring (device values only gate
    candidate *coverage*, which has large margin).
  - DMA engine parallelism: X loads are issued from SP, entry/raw exports
    from the (otherwise idle) GPSIMD engine; transfers occupy the issuing
    engine's timeline, so they overlap compute on ACT/DVE/PE.
"""

import numpy as np
import ml_dtypes

BF16 = ml_dtypes.bfloat16

B, D, N, NQ, K = 256, 128, 200000, 10000, 50
NCORES = 8
SHARD = 25000
SEG = 2048                    # columns per instance (4 PSUM banks f32)
NSEG = 13
PADDED = NSEG * SEG           # 26624
W = 1024                      # columns pair-merged by DVE per instance
NINST = 2 * NSEG              # 26 instances (2 query groups x 13 segments)
TAU = 0.1
EPS = 1e-8
ALPHA, BETA, LAMB, GAMMA = 1.0, 1.0, 1e-4, 1.0

# host-side selection breadth
T_ENT = 3100                  # top entries taken per row (each -> 2 cols)

_cache = {}
last_results = None


def _patch_tail_drain():
    """Split the TileContext tail drain into one drain per pending proc:
    the stock implementation attaches a wait for EVERY proc in the global
    clock to a single Drain, overflowing the ISA's sync-wait slots."""
    import concourse.tile as tile
    from concourse.vector_clock import ScopedClock, VectorClock

    if getattr(tile.TileContext, "_ant_split_drain", False):
        return

    def _drain_and_barrier(self, tick_clock, wait_clock):
        vc = tick_clock.global_clock
        for proc in range(len(vc)):
            t = vc[proc]
            if t > 0:
                drain_inst = self.nc.sync.drain()
                sub = [0] * len(vc)
                sub[proc] = t
                wait_clock.add_sem_waits(
                    drain_inst.ins, ScopedClock({None: VectorClock(sub)})
                )
        self.nc.all_engine_barrier()
        assert self.sems is not None
        popped = self.nc._tile_sem_poison_stack.pop()
        assert popped is self._sem_poison
        self.nc.clear_and_free_semaphores(list(self.sems.allocated().values()))
        self.nc.all_engine_barrier()

    tile.TileContext._drain_and_barrier = _drain_and_barrier
    tile.TileContext._ant_split_drain = True


def _split_multi_waits(nc, max_waits=1):
    """Walrus legality pass: TRN2 instruction structs carry very few sync-wait
    slots (1 for Matmult/DMA/Activation/TensorTensor).  Hoist excess waits
    onto same-engine NoOps inserted right before the instruction — the engine
    queue stalls on the NoOp first, preserving semantics exactly."""
    import concourse.mybir as mybir
    f = nc.m.functions[0]
    for blk in f.blocks:
        insts = blk.instructions
        out = []
        changed = False
        for inst in insts:
            si = getattr(inst, "sync_info", None)
            if si is not None and len(si.on_wait) > max_waits:
                waits = list(si.on_wait)
                for w in waits[:-max_waits]:
                    nop = mybir.InstNoOp(name=f"I-wsplit-{nc.next_id()}")
                    nop.engine = inst.engine
                    nop.sync_info = mybir.SyncInfo(on_wait=[w], on_update=[])
                    out.append(nop)
                inst.sync_info = mybir.SyncInfo(
                    on_wait=waits[-max_waits:], on_update=list(si.on_update))
                changed = True
            out.append(inst)
        if changed:
            blk.instructions = out


def _build_bass(trace_sim=False):
    import concourse.bass as bass
    import concourse.mybir as mybir
    from concourse.tile import TileContext

    _patch_tail_drain()

    nc = bass.Bass()
    lhs_d = nc.dram_tensor("lhs", [128, 256], mybir.dt.bfloat16, kind="ExternalInput")
    xt_d = nc.dram_tensor("xt", [128, PADDED], mybir.dt.bfloat16, kind="ExternalInput")
    ent_d = nc.dram_tensor("ent", [128, NINST * W], mybir.dt.bfloat16,
                           kind="ExternalOutput")

    with TileContext(nc, trace_sim=trace_sim) as tc:
        with (
            tc.tile_pool(name="xin", bufs=1) as xin_pool,
            tc.tile_pool(name="ps", bufs=1, space="PSUM") as psum_pool,
            tc.tile_pool(name="misc", bufs=1) as misc_pool,
        ):
            lhs_sb = misc_pool.tile([128, 256], mybir.dt.bfloat16, tag="lhs")
            nc.sync.dma_start(out=lhs_sb[:], in_=lhs_d[:])
            xt_tiles = [xin_pool.tile([128, SEG], mybir.dt.bfloat16,
                                      name=f"xt{s}", tag=f"xt{s}")
                        for s in range(NSEG)]
            for s in range(NSEG):
                nc.sync.dma_start(out=xt_tiles[s][:],
                                  in_=xt_d[:, s * SEG:(s + 1) * SEG])
            # Split each parity's PSUM into two 1024-col tiles: psA is read
            # only by DVE (tensor_max in0), psB only by ACT (staging copy).
            # The WAR chains then decouple per engine: matmuls into psB wait
            # on ACT alone, matmuls into psA on DVE alone, so ACT/DVE never
            # round-robin behind each other.
            psA = [psum_pool.tile([128, W], mybir.dt.float32,
                                  name=f"psA{i}", tag=f"psA{i}") for i in range(2)]
            psB = [psum_pool.tile([128, W], mybir.dt.float32,
                                  name=f"psB{i}", tag=f"psB{i}") for i in range(2)]
            stg_all = misc_pool.tile([128, NINST * W], mybir.dt.bfloat16, tag="stg")
            ent_all = misc_pool.tile([128, NINST * W], mybir.dt.bfloat16, tag="ent")

            for i in range(NINST):
                s, g = i // 2, i % 2
                xt_sb = xt_tiles[s]
                # B-half first so ACT can start while A-half matmuls run
                for m in range(2):
                    nc.tensor.matmul(
                        psB[g][:, m * 512:(m + 1) * 512],
                        lhs_sb[:, g * 128:(g + 1) * 128],
                        xt_sb[:, W + m * 512:W + (m + 1) * 512],
                        start=True, stop=True)
                for m in range(2):
                    nc.tensor.matmul(
                        psA[g][:, m * 512:(m + 1) * 512],
                        lhs_sb[:, g * 128:(g + 1) * 128],
                        xt_sb[:, m * 512:(m + 1) * 512],
                        start=True, stop=True)
                o = i * W
                nc.scalar.copy(stg_all[:, o:o + W], psB[g][:])
                nc.vector.tensor_max(ent_all[:, o:o + W],
                                     psA[g][:], stg_all[:, o:o + W])
                if g == 1:
                    nc.gpsimd.dma_start(
                        out=ent_d[:, (i - 1) * W:(i + 1) * W],
                        in_=ent_all[:, (i - 1) * W:(i + 1) * W])
    _split_multi_waits(nc)
    return nc


def _device_scores(Tq32, X32, xsq64):
    """Run the 8-core SPMD kernel; return per-core (ent, raw) bf16 arrays."""
    global last_results
    from concourse.bass_utils import run_bass_kernel_spmd

    if "nc" not in _cache:
        _cache["nc"] = _build_bass()
    nc = _cache["nc"]

    lhs = np.zeros([128, 256], np.float32)
    lhs[:127, :] = Tq32.T[:127, :]
    lhs[127, :] = 1.0
    lhs = lhs.astype(BF16)

    in_maps = []
    for c in range(NCORES):
        xt = np.zeros([128, PADDED], np.float32)
        sl = X32[c * SHARD:(c + 1) * SHARD]
        xt[:127, :SHARD] = sl.T[:127, :]
        xt[127, :SHARD] = (-0.5 * xsq64[c * SHARD:(c + 1) * SHARD]).astype(np.float32)
        xt[127, SHARD:] = -1e30
        in_maps.append({"lhs": lhs, "xt": xt.astype(BF16)})

    import time
    t0 = time.perf_counter()
    last_results = run_bass_kernel_spmd(nc, in_maps, core_ids=list(range(NCORES)))
    _cache["spmd_wall_s"] = time.perf_counter() - t0
    results = last_results.results

    ents = [np.asarray(r["ent"]) for r in results]   # [128, NINST*W] bf16
    return ents


def _topk_exact(Tq64, X64, ents, k=K):
    """Exact top-k per row from device candidate scores.  Entry (s, j)
    covers core-local cols {s*SEG + j, s*SEG + W + j}."""
    s = np.arange(NSEG)
    j = np.arange(W)
    e0 = (s[:, None] * SEG + j[None, :]).ravel()          # [NSEG*W]
    e1 = e0 + W
    # per-core global column offsets; pad locals (>= SHARD) masked per map
    ecols0 = np.concatenate([c * SHARD + e0 for c in range(NCORES)])
    ecols1 = np.concatenate([c * SHARD + e1 for c in range(NCORES)])
    e0valid = np.tile(e0 < SHARD, NCORES)
    e1valid = np.tile(e1 < SHARD, NCORES)
    evalid = e0valid | e1valid

    out = np.empty((B, k), np.int64)
    for q in range(B):
        g, p = divmod(q, 128)
        ev = np.concatenate(
            [e[p].reshape(NINST, W)[g::2].ravel() for e in ents]).astype(np.float32)
        ev[~evalid] = -np.inf
        te = np.argpartition(-ev, T_ENT)[:T_ENT]
        cc = np.concatenate([ecols0[te][e0valid[te]],
                             ecols1[te][e1valid[te]]])
        cc = np.unique(cc)
        diff = X64[cc] - Tq64[q]
        d2 = np.einsum("ij,ij->i", diff, diff)
        order = np.lexsort((cc, d2))
        out[q] = cc[order[:k]]
    return out


def _sqdist(A, Bm):
    d2 = (A * A).sum(1)[:, None] + (Bm * Bm).sum(1)[None, :] - 2.0 * (A @ Bm.T)
    return np.maximum(d2, 0.0)


def _host_loss(q_batch, X, Wm, b, pre_weights, pre_indices, q_indices, idx, post_idx):
    """Mirror of reference() in numpy f64, given the KNN indices."""
    Tq = q_batch @ Wm.T + b
    s, t = Tq, X[idx]
    comb = np.concatenate([s, t], 0)
    sigma_sq = np.median(_sqdist(comb, comb)) / 2.0
    if sigma_sq < 1e-6:
        sigma_sq = 1.0
    g = 1.0 / (sigma_sq + EPS)
    kxx = np.exp(-g * _sqdist(s, s)).mean()
    kyy = np.exp(-g * _sqdist(t, t)).mean()
    kxy = np.exp(-g * _sqdist(s, t)).mean()
    loss_dist = max(kxx + kyy - 2.0 * kxy, 0.0)
    Xn = X[post_idx]                                   # [B, K, d]
    l2 = ((Tq[:, None, :] - Xn) ** 2).sum(-1)          # [B, K]
    z = -l2 / TAU
    z = z - z.max(1, keepdims=True)
    ez = np.exp(z)
    post_w = ez / ez.sum(1, keepdims=True)
    pre_i = pre_indices[q_indices]                     # [B, K]
    pre_w = pre_weights[q_indices]                     # [B, K]
    cat = np.concatenate([pre_i, post_idx], axis=1)    # [B, 2K]
    mult = (cat[:, :, None] == cat[:, None, :]).sum(-1).astype(np.float64)
    p_raw = np.einsum("bmk,bk->bm",
                      (cat[:, :, None] == pre_i[:, None, :]).astype(np.float64), pre_w)
    q_raw = np.einsum("bmk,bk->bm",
                      (cat[:, :, None] == post_idx[:, None, :]).astype(np.float64), post_w)
    p_c = np.maximum(p_raw, EPS)
    q_c = np.maximum(q_raw, EPS)
    p = p_c / (p_c / mult).sum(1, keepdims=True)
    q = q_c / (q_c / mult).sum(1, keepdims=True)
    kl = ((p * (np.log(p) - np.log(q))) / mult).sum(1)
    loss_knn = kl.mean()
    loss_reg = 0.5 * ((Wm ** 2).sum() + (b ** 2).sum())
    loss_anchor = ((Tq - q_batch) ** 2).sum(1).mean()
    total = ALPHA * loss_dist + BETA * loss_knn + LAMB * loss_reg + GAMMA * loss_anchor
    return np.stack([total, loss_dist, loss_knn, loss_anchor]).astype(np.float32)


def kernel(q_batch, X, W, b, pre_weights, pre_indices, q_indices, idx):
    q_batch = np.asarray(q_batch, np.float32)
    X32 = np.asarray(X, np.float32)
    W32 = np.asarray(W, np.float32)
    b32 = np.asarray(b, np.float32)
    pre_weights = np.asarray(pre_weights, np.float64)
    pre_indices = np.asarray(pre_indices, np.int64)
    q_indices = np.asarray(q_indices, np.int64)
    idx = np.asarray(idx, np.int64)

    Tq32 = q_batch @ W32.T + b32
    X64 = X32.astype(np.float64)
    Tq64 = Tq32.astype(np.float64)
    xsq64 = (X64 * X64).sum(1)

    ents = _device_scores(Tq32, X32, xsq64)
    post_idx = _topk_exact(Tq64, X64, ents)

    return _host_loss(q_batch.astype(np.float64), X64, W32.astype(np.float64),
                      b32.astype(np.float64), pre_weights, pre_indices,
                      q_indices, idx, post_idx)
